# revision 32
# baseline (speedup 1.0000x reference)
"""Trainium2 Bass kernel for nn_ClusterMemory_47923245088802.

Computes: loss = mean_b( logsumexp_n(<x_b/||x_b||, f_n>/temp) - <x_b/||x_b||, f_{t_b}>/temp )
with x [4096,1024], f [32768,1024] (rows ~unit norm), t = corrected_targets.

Sharding: features rows split across 8 cores (4096 each, tensor parallel over
num_samples). Each core computes its [4096 x 4096] logit block on the PE array
in fp8-e4m3 DoubleRow mode and reduces it with exp + row-accumulate on the
scalar engine; the host combines the 8 partial sum-exps with a log (the
cross-shard all-reduce of the CE log-sum-exp).

x is L2-normalized on the host and both operands are pre-scaled by 64 to clear
the e4m3 subnormal band (the 1/64^2 is folded into the constant exp scale), so
the device kernel is a pure matmul->exp->accumulate stream: no norm phase, no
per-row scale, no target-dot matmuls (the 4096 target dots are exact host f32).

Layout: ko-parity-major [P, 2, K2=4, 512] slices. The DoubleRow pair dim gets
a 2048B stride (512B strides slow the PE's dual-stream SBUF reads to 259 vs
215.5 ns per matmul; 2048B and 4096B measured full-speed) and every input DMA
is a whole [128 x 4KB]-contiguous slice (host pre-tiled; big descriptors move
much faster through a queue than sub-2KB ones). Slices are issued in
consumption-deadline order on the sync and gpsimd queues; quarter-width groups
for the first 4 batch tiles mean only x0+f0 (1MB) gate the stream start
(~12us), and half-width groups for the last 2 tiles keep the final ACT short.
~6.4us of junk warmup matmuls hold the PE's HAM clock-gate open until data
lands; quarter groups (and tile 30's halves) row-sum on the vector engine
instead of accum_out, whose READ_ACCUMULATOR serializes the ACT chain. Matmul stream measures 215.5 ns/matmul (the N=512 fp8-DR issue floor).
Occasionally a run lands ~20% slower wholesale (P0 power-state downclock to
2.0 GHz) — rerun, don't chase phantom regressions.
"""

import numpy as np
import ml_dtypes

B = 4096          # batch
D = 1024          # feature dim (contraction)
NTOT = 32768      # num_samples
TEMP = 0.05
NCORES = 8
NS = NTOT // NCORES   # samples per core
P = 128
KO = D // P           # 8 k-chunks
K2 = KO // 2          # 4 DoubleRow k-chunk pairs
BT = B // P           # 32 batch tiles
NSL = 8               # 512-column slices of x and of f
SCALE = 64.0          # host pre-scale on x and f before e4m3 quantization
ESCALE = 1.0 / (SCALE * SCALE * TEMP)   # exp scale: dot -> logits

_CACHE = {}


def _build_nc():
    from contextlib import ExitStack

    import concourse.bass as bass
    import concourse.bacc as bacc
    import concourse.mybir as mybir
    import concourse.tile as tile

    f32 = mybir.dt.float32
    fp8 = mybir.dt.float8e4
    AF = mybir.ActivationFunctionType
    DR = mybir.MatmulPerfMode.DoubleRow

    nc = bacc.Bacc("TRN2", target_bir_lowering=False, debug=False,
                   enable_asserts=False)

    xt = nc.dram_tensor("xt", [NSL, P, 2, K2, 512], fp8, kind="ExternalInput")
    ft = nc.dram_tensor("ft", [NSL, P, 2, K2, 512], fp8, kind="ExternalInput")
    # accum columns: 0..31 per-tile [i, jj]; 32: tile-31 extras; 33: tile-30
    # extras; 34..37: tiles 0-3 quarter j=1; 38..41: tiles 0-3 half h1;
    # 42..45: tiles 4-7 half h1.
    sacc_out = nc.dram_tensor("sacc", [P, BT + 14, 2], f32, kind="ExternalOutput")

    with tile.TileContext(nc) as tc, ExitStack() as ctx:
        io = ctx.enter_context(tc.tile_pool(name="io", bufs=1))
        stats = ctx.enter_context(tc.tile_pool(name="stats", bufs=1))

        x_sb = [io.tile([P, 2, K2, 512], fp8, name=f"xs{j}") for j in range(NSL)]
        f_sb = [io.tile([P, 2, K2, 512], fp8, name=f"fs{j}") for j in range(NSL)]

        # Input DMAs in consumption-deadline order. x0+f0 first (they alone
        # gate the quarter-group stream start) serial on sync, whose queue
        # starts ~3us before gpsimd's and bursts well above fair-share while
        # alone; the scalar queue carries nothing (it starves once ACTIVATEs
        # run).
        nc.sync.dma_start(x_sb[0][:], xt.ap()[0])
        nc.sync.dma_start(f_sb[0][:], ft.ap()[0])
        nc.sync.dma_start(f_sb[1][:], ft.ap()[1])
        nc.gpsimd.dma_start(f_sb[2][:], ft.ap()[2])
        nc.sync.dma_start(x_sb[1][:], xt.ap()[1])
        nc.gpsimd.dma_start(f_sb[3][:], ft.ap()[3])
        nc.sync.dma_start(f_sb[5][:], ft.ap()[5])
        nc.gpsimd.dma_start(f_sb[4][:], ft.ap()[4])
        nc.sync.dma_start(f_sb[7][:], ft.ap()[7])
        nc.gpsimd.dma_start(f_sb[6][:], ft.ap()[6])
        nc.sync.dma_start(x_sb[3][:], xt.ap()[3])
        nc.gpsimd.dma_start(x_sb[2][:], xt.ap()[2])
        nc.sync.dma_start(x_sb[5][:], xt.ap()[5])
        nc.gpsimd.dma_start(x_sb[4][:], xt.ap()[4])
        nc.sync.dma_start(x_sb[7][:], xt.ap()[7])
        nc.gpsimd.dma_start(x_sb[6][:], xt.ap()[6])

        # Preload the exp table on the scalar engine during the DMA window:
        # a junk 8-element exp forces walrus's ACT_TABLE_LOAD here instead of
        # in front of the first real (on-critical-path) activation.
        junk = stats.tile([P, 8], f32)
        nc.scalar.activation(junk[:], junk[:], AF.Exp, bias=0.0, scale=0.0)

        sacc_all = stats.tile([P, BT + 14, 2], f32)
        dummy = stats.tile([P, 2048], f32)    # unused act main output
        wz = stats.tile([P, 512], fp8)        # zeros for HAM warmup matmuls
        nc.vector.memset(wz[:], 0.0)

        # Main loop: [4096 x 4096] logits in fp8 DoubleRow, exp + row-sum.
        # Steady state: 4 n-slices share one 4-bank psum tile so a single
        # wide ACTIVATE covers 2048 columns (amortizes the ACT overhead).
        with tc.tile_pool(name="psm", bufs=2, space="PSUM") as psm:
            # HAM warmup: the PE clock-gate defaults to 1.2 GHz and needs
            # ~3.4us of sustained activity to release to 2.4 GHz. The PE is
            # idle waiting for the first DMAs anyway; burn that window on
            # junk matmuls over a zeroed tile.
            pw = psm.tile([P, 4, 512], f32, name="pl")
            for w in range(15):
                nc.tensor.matmul(pw[:, w % 4, :], wz[:, :P], wz[:],
                                 start=True, stop=True)

            def mms(pl, i, j2s):
                xw = x_sb[i // 4]
                q = P * (i % 4)
                # k2-major: weight reuse across the n-slices of the group
                for k2 in range(K2):
                    for gi, j in enumerate(j2s):
                        nc.tensor.matmul(
                            pl[:, gi, :],
                            xw[:, :, k2, q:q + P],
                            f_sb[j][:, :, k2, :],
                            start=k2 == 0, stop=k2 == K2 - 1,
                            perf_mode=DR)

            qrot = [0]
            hrot = [0]

            def emit(i, j2s, col, jj, vred=False):
                pl = psm.tile([P, 4, 512], f32, name="pl")
                mms(pl, i, j2s)
                n = 512 * len(j2s)
                if n == 512:
                    # Quarter groups: accum_out's serializing
                    # READ_ACCUMULATOR would make the ACT chain (1.0us)
                    # outpace the 0.86us matmul groups and stall the psum
                    # rotation. Instead: plain exp into a rotating dummy
                    # slice, row-sum on the otherwise-idle vector engine.
                    off = 512 * (qrot[0] % 4)
                    qrot[0] += 1
                    nc.scalar.activation(dummy[:, off:off + 512],
                                         pl[:, :1, :], AF.Exp, bias=0.0,
                                         scale=ESCALE)
                    nc.vector.reduce_sum(sacc_all[:, col, jj:jj + 1],
                                         dummy[:, off:off + 512],
                                         axis=mybir.AxisListType.X)
                elif vred:
                    # Same trick for tile 30's half groups: their 1712ns
                    # accum ACT chain sits at exact parity with the 1725ns
                    # matmul groups and jitter-stalls the psum rotation.
                    off = 1024 * (hrot[0] % 2)
                    hrot[0] += 1
                    nc.scalar.activation(dummy[:, off:off + 1024],
                                         pl[:, :2, :], AF.Exp, bias=0.0,
                                         scale=ESCALE)
                    nc.vector.reduce_sum(sacc_all[:, col, jj:jj + 1],
                                         dummy[:, off:off + 1024],
                                         axis=mybir.AxisListType.X)
                else:
                    nc.scalar.activation(dummy[:, :n], pl[:, :len(j2s), :],
                                         AF.Exp, bias=0.0, scale=ESCALE,
                                         accum_out=sacc_all[:, col, jj:jj + 1])

            # Quarter-groups for tiles 0..3: only x0+f0 (1MB) gate the
            # stream start; f1 isn't needed until ~3.5us later.
            for i in range(4):
                emit(i, (0,), i, 0)
            for i in range(4):
                emit(i, (1,), 34 + i, 0)
            # Half-groups while the DMA front catches up.
            for i in range(4, 8):
                emit(i, (0, 1), i, 0)
            for i in range(4):
                emit(i, (2, 3), 38 + i, 0)
            for i in range(4, 8):
                emit(i, (2, 3), 42 + i - 4, 0)
            # Steady state: full-width groups.
            for i in range(8):
                emit(i, (4, 5, 6, 7), i, 1)
            for i in range(8, BT - 2):
                emit(i, (0, 1, 2, 3), i, 0)
                emit(i, (4, 5, 6, 7), i, 1)
            # Last two tiles in half-groups: a 1024-wide ACT keeps up with
            # its 8-matmul group, so only one short ACT rides the
            # end-of-kernel critical path.
            i = BT - 2
            emit(i, (0, 1), i, 0, vred=True)
            emit(i, (2, 3), 33, 0, vred=True)
            emit(i, (4, 5), i, 1, vred=True)
            emit(i, (6, 7), 33, 1, vred=True)
            nc.sync.dma_start(sacc_out.ap()[:, :BT - 1], sacc_all[:, :BT - 1])
            nc.gpsimd.dma_start(sacc_out.ap()[:, 33:], sacc_all[:, 33:])
            i = BT - 1
            emit(i, (0, 1), i, 0)
            emit(i, (2, 3), 32, 0)
            emit(i, (4, 5), i, 1)
            emit(i, (6, 7), 32, 1)

        nc.sync.dma_start(sacc_out.ap()[:, BT - 1:33], sacc_all[:, BT - 1:33])

    nc.compile()
    return nc


def _get_nc():
    if "nc" not in _CACHE:
        _CACHE["nc"] = _build_nc()
    return _CACHE["nc"]


def _tile_slices(aT):
    """[D, N] (d-major) -> [NSL, P, 2, K2, 512] ko-parity-major slices.

    out[s, p, r, k2, b] = aT[(2*k2 + r)*128 + p, 512*s + b]
    """
    n = aT.shape[1]
    a = aT.reshape(K2, 2, P, n // 512, 512)        # [k2, r, p, s, b]
    return np.ascontiguousarray(a.transpose(3, 2, 1, 0, 4))


def _prep(inputs, corrected_targets, features):
    import concourse.mybir as mybir
    fp8 = mybir.dt.np(mybir.dt.float8e4)
    x = np.asarray(inputs, dtype=np.float32)
    f = np.asarray(features, dtype=np.float32)
    ct = np.asarray(corrected_targets).astype(np.int64)

    xn = x / np.linalg.norm(x, axis=1, keepdims=True)
    tdot = np.einsum('bd,bd->b', xn.astype(np.float64),
                     f[ct].astype(np.float64)) / TEMP

    xt = _tile_slices(np.ascontiguousarray((xn * SCALE).T)).astype(fp8)
    fT = np.ascontiguousarray((f * SCALE).T)                  # [D, NTOT]
    in_maps = []
    for c in range(NCORES):
        in_maps.append({
            "xt": xt,
            "ft": _tile_slices(fT[:, c * NS:(c + 1) * NS]).astype(fp8),
        })
    return in_maps, tdot


def _combine(results, tdot):
    S = np.zeros(B, dtype=np.float64)
    for c in range(NCORES):
        sacc = results[c]["sacc"].astype(np.float64)
        part = sacc[:, :BT].sum(axis=2)          # [P, BT]
        # partial-group accums (all jj=0 slots of their extra columns):
        # tiles 0-3: quarter j=1 in 34..37, half h1 in 38..41;
        # tiles 4-7: half h1 in 42..45; tiles 30/31 extras in 33/32 (both jj).
        part[:, 0:4] += sacc[:, 34:38, 0] + sacc[:, 38:42, 0]
        part[:, 4:8] += sacc[:, 42:46, 0]
        part[:, BT - 2] += sacc[:, 33, :].sum(axis=1)
        part[:, BT - 1] += sacc[:, 32, :].sum(axis=1)
        S += part.T.ravel()
    loss = np.mean(np.log(S) - tdot)
    return np.asarray(loss, dtype=np.float32)


def _run(inputs, targets, corrected_targets, features, trace=False, tmpdir=None):
    import time
    from concourse import bass_utils
    nc = _get_nc()
    in_maps, tdot = _prep(inputs, corrected_targets, features)
    last_exc = None
    for attempt in range(3):
        try:
            res = bass_utils.run_bass_kernel_spmd(
                nc, in_maps, core_ids=list(range(NCORES)), trace=trace,
                tmpdir=tmpdir)
            return _combine(res.results, tdot), res
        except Exception as e:  # transient device state (e.g. prior crash)
            last_exc = e
            time.sleep(2.0)
    raise last_exc


def kernel(inputs, targets, corrected_targets, features):
    out, _ = _run(inputs, targets, corrected_targets, features, trace=False)
    return out


# revision 34
# speedup vs baseline: 1.0027x; 1.0027x over previous
"""Trainium2 Bass kernel for nn_ClusterMemory_47923245088802.

Computes: loss = mean_b( logsumexp_n(<x_b/||x_b||, f_n>/temp) - <x_b/||x_b||, f_{t_b}>/temp )
with x [4096,1024], f [32768,1024] (rows ~unit norm), t = corrected_targets.

Sharding: features rows split across 8 cores (4096 each, tensor parallel over
num_samples). Each core computes its [4096 x 4096] logit block on the PE array
in fp8-e4m3 DoubleRow mode and reduces it with exp + row-accumulate on the
scalar engine; the host combines the 8 partial sum-exps with a log (the
cross-shard all-reduce of the CE log-sum-exp).

x is L2-normalized on the host and both operands are pre-scaled by 64 to clear
the e4m3 subnormal band (the 1/64^2 is folded into the constant exp scale), so
the device kernel is a pure matmul->exp->accumulate stream: no norm phase, no
per-row scale, no target-dot matmuls (the 4096 target dots are exact host f32).

Layout: ko-parity-major [P, 2, K2=4, 512] slices. The DoubleRow pair dim gets
a 2048B stride (512B strides slow the PE's dual-stream SBUF reads to 259 vs
215.5 ns per matmul; 2048B and 4096B measured full-speed) and every input DMA
is a whole [128 x 4KB]-contiguous slice (host pre-tiled; big descriptors move
much faster through a queue than sub-2KB ones). Slices are issued in
consumption-deadline order on the sync and gpsimd queues; quarter-width groups
for the first 4 batch tiles mean only x0+f0 (1MB) gate the stream start
(~12us), and half-width groups for the last 2 tiles keep the final ACT short.
~6.4us of junk warmup matmuls hold the PE's HAM clock-gate open until data
lands; quarter groups (and tile 30's halves) row-sum on the vector engine
instead of accum_out, whose READ_ACCUMULATOR serializes the ACT chain. Matmul stream measures 215.5 ns/matmul (the N=512 fp8-DR issue floor).
Occasionally a run lands ~20% slower wholesale (P0 power-state downclock to
2.0 GHz) — rerun, don't chase phantom regressions.
"""

import numpy as np
import ml_dtypes

B = 4096          # batch
D = 1024          # feature dim (contraction)
NTOT = 32768      # num_samples
TEMP = 0.05
NCORES = 8
NS = NTOT // NCORES   # samples per core
P = 128
KO = D // P           # 8 k-chunks
K2 = KO // 2          # 4 DoubleRow k-chunk pairs
BT = B // P           # 32 batch tiles
NSL = 8               # 512-column slices of x and of f
SCALE = 64.0          # host pre-scale on x and f before e4m3 quantization
ESCALE = 1.0 / (SCALE * SCALE * TEMP)   # exp scale: dot -> logits

_CACHE = {}


def _build_nc():
    from contextlib import ExitStack

    import concourse.bass as bass
    import concourse.bacc as bacc
    import concourse.mybir as mybir
    import concourse.tile as tile

    f32 = mybir.dt.float32
    fp8 = mybir.dt.float8e4
    AF = mybir.ActivationFunctionType
    DR = mybir.MatmulPerfMode.DoubleRow

    nc = bacc.Bacc("TRN2", target_bir_lowering=False, debug=False,
                   enable_asserts=False)

    xt = nc.dram_tensor("xt", [NSL, P, 2, K2, 512], fp8, kind="ExternalInput")
    ft = nc.dram_tensor("ft", [NSL, P, 2, K2, 512], fp8, kind="ExternalInput")
    # accum columns: 0..31 per-tile [i, jj]; 32: tile-31 extras; 33: tile-30
    # extras; 34..37: tiles 0-3 quarter j=1; 38..41: tiles 0-3 half h1;
    # 42..45: tiles 4-7 half h1.
    sacc_out = nc.dram_tensor("sacc", [P, BT + 14, 2], f32, kind="ExternalOutput")

    with tile.TileContext(nc) as tc, ExitStack() as ctx:
        io = ctx.enter_context(tc.tile_pool(name="io", bufs=1))
        stats = ctx.enter_context(tc.tile_pool(name="stats", bufs=1))

        x_sb = [io.tile([P, 2, K2, 512], fp8, name=f"xs{j}") for j in range(NSL)]
        f_sb = [io.tile([P, 2, K2, 512], fp8, name=f"fs{j}") for j in range(NSL)]

        # Input DMAs in consumption-deadline order. x0+f0 first (they alone
        # gate the quarter-group stream start) serial on sync, whose queue
        # starts ~3us before gpsimd's and bursts well above fair-share while
        # alone; the scalar queue carries nothing (it starves once ACTIVATEs
        # run).
        nc.sync.dma_start(x_sb[0][:], xt.ap()[0])
        nc.sync.dma_start(f_sb[0][:], ft.ap()[0])
        nc.sync.dma_start(f_sb[1][:], ft.ap()[1])
        nc.gpsimd.dma_start(f_sb[2][:], ft.ap()[2])
        nc.sync.dma_start(x_sb[1][:], xt.ap()[1])
        nc.gpsimd.dma_start(f_sb[3][:], ft.ap()[3])
        nc.sync.dma_start(f_sb[5][:], ft.ap()[5])
        nc.gpsimd.dma_start(f_sb[4][:], ft.ap()[4])
        nc.sync.dma_start(f_sb[7][:], ft.ap()[7])
        nc.gpsimd.dma_start(f_sb[6][:], ft.ap()[6])
        nc.sync.dma_start(x_sb[3][:], xt.ap()[3])
        nc.gpsimd.dma_start(x_sb[2][:], xt.ap()[2])
        nc.sync.dma_start(x_sb[5][:], xt.ap()[5])
        nc.gpsimd.dma_start(x_sb[4][:], xt.ap()[4])
        nc.sync.dma_start(x_sb[7][:], xt.ap()[7])
        nc.gpsimd.dma_start(x_sb[6][:], xt.ap()[6])

        # Preload the exp table on the scalar engine during the DMA window:
        # a junk 8-element exp forces walrus's ACT_TABLE_LOAD here instead of
        # in front of the first real (on-critical-path) activation.
        junk = stats.tile([P, 8], f32)
        nc.scalar.activation(junk[:], junk[:], AF.Exp, bias=0.0, scale=0.0)

        sacc_all = stats.tile([P, BT + 14, 2], f32)
        dummy = stats.tile([P, 2048], f32)    # unused act main output
        vdummy = stats.tile([P, 4096], f32)   # act out for vector-reduced fulls
        wz = stats.tile([P, 512], fp8)        # zeros for HAM warmup matmuls
        nc.vector.memset(wz[:], 0.0)

        # Main loop: [4096 x 4096] logits in fp8 DoubleRow, exp + row-sum.
        # Steady state: 4 n-slices share one 4-bank psum tile so a single
        # wide ACTIVATE covers 2048 columns (amortizes the ACT overhead).
        with tc.tile_pool(name="psm", bufs=2, space="PSUM") as psm:
            # HAM warmup: the PE clock-gate defaults to 1.2 GHz and needs
            # ~3.4us of sustained activity to release to 2.4 GHz. The PE is
            # idle waiting for the first DMAs anyway; burn that window on
            # junk matmuls over a zeroed tile.
            pw = psm.tile([P, 4, 512], f32, name="pl")
            for w in range(15):
                nc.tensor.matmul(pw[:, w % 4, :], wz[:, :P], wz[:],
                                 start=True, stop=True)

            def mms(pl, i, j2s):
                xw = x_sb[i // 4]
                q = P * (i % 4)
                # k2-major: weight reuse across the n-slices of the group
                for k2 in range(K2):
                    for gi, j in enumerate(j2s):
                        nc.tensor.matmul(
                            pl[:, gi, :],
                            xw[:, :, k2, q:q + P],
                            f_sb[j][:, :, k2, :],
                            start=k2 == 0, stop=k2 == K2 - 1,
                            perf_mode=DR)

            qrot = [0]
            hrot = [0]

            def emit(i, j2s, col, jj, vred=False):
                pl = psm.tile([P, 4, 512], f32, name="pl")
                mms(pl, i, j2s)
                n = 512 * len(j2s)
                if n == 512:
                    # Quarter groups: accum_out's serializing
                    # READ_ACCUMULATOR would make the ACT chain (1.0us)
                    # outpace the 0.86us matmul groups and stall the psum
                    # rotation. Instead: plain exp into a rotating dummy
                    # slice, row-sum on the otherwise-idle vector engine.
                    off = 512 * (qrot[0] % 4)
                    qrot[0] += 1
                    nc.scalar.activation(dummy[:, off:off + 512],
                                         pl[:, :1, :], AF.Exp, bias=0.0,
                                         scale=ESCALE)
                    nc.vector.reduce_sum(sacc_all[:, col, jj:jj + 1],
                                         dummy[:, off:off + 512],
                                         axis=mybir.AxisListType.X)
                elif vred and n == 1024:
                    # Same trick for tile 30's half groups: their 1712ns
                    # accum ACT chain sits at exact parity with the 1725ns
                    # matmul groups and jitter-stalls the psum rotation.
                    off = 1024 * (hrot[0] % 2)
                    hrot[0] += 1
                    nc.scalar.activation(dummy[:, off:off + 1024],
                                         pl[:, :2, :], AF.Exp, bias=0.0,
                                         scale=ESCALE)
                    nc.vector.reduce_sum(sacc_all[:, col, jj:jj + 1],
                                         dummy[:, off:off + 1024],
                                         axis=mybir.AxisListType.X)
                elif vred:
                    # Tile 29's FULL groups: with accum_out, the last full
                    # group's psum-slot release includes its 283ns
                    # READ_ACCUMULATOR (2.25us total) while the following
                    # half groups reuse the slot after 1.72us of matmuls ->
                    # 0.9us boundary stall. ACT-only release (1.97us) into a
                    # dedicated scratch (so later half ACTs never wait on the
                    # vector reduces) cuts it to ~0.25us.
                    off = 2048 * (hrot[0] % 2)
                    hrot[0] += 1
                    nc.scalar.activation(vdummy[:, off:off + 2048],
                                         pl[:], AF.Exp, bias=0.0,
                                         scale=ESCALE)
                    nc.vector.reduce_sum(sacc_all[:, col, jj:jj + 1],
                                         vdummy[:, off:off + 2048],
                                         axis=mybir.AxisListType.X)
                else:
                    nc.scalar.activation(dummy[:, :n], pl[:, :len(j2s), :],
                                         AF.Exp, bias=0.0, scale=ESCALE,
                                         accum_out=sacc_all[:, col, jj:jj + 1])

            # Quarter-groups for tiles 0..3: only x0+f0 (1MB) gate the
            # stream start; f1 isn't needed until ~3.5us later.
            for i in range(4):
                emit(i, (0,), i, 0)
            for i in range(4):
                emit(i, (1,), 34 + i, 0)
            # Half-groups while the DMA front catches up.
            for i in range(4, 8):
                emit(i, (0, 1), i, 0)
            for i in range(4):
                emit(i, (2, 3), 38 + i, 0)
            for i in range(4, 8):
                emit(i, (2, 3), 42 + i - 4, 0)
            # Steady state: full-width groups.
            for i in range(8):
                emit(i, (4, 5, 6, 7), i, 1)
            for i in range(8, BT - 3):
                emit(i, (0, 1, 2, 3), i, 0)
                emit(i, (4, 5, 6, 7), i, 1)
            i = BT - 3
            emit(i, (0, 1, 2, 3), i, 0, vred=True)
            emit(i, (4, 5, 6, 7), i, 1, vred=True)
            # Last two tiles in half-groups: a 1024-wide ACT keeps up with
            # its 8-matmul group, so only one short ACT rides the
            # end-of-kernel critical path.
            i = BT - 2
            emit(i, (0, 1), i, 0, vred=True)
            emit(i, (2, 3), 33, 0, vred=True)
            emit(i, (4, 5), i, 1, vred=True)
            emit(i, (6, 7), 33, 1, vred=True)
            nc.sync.dma_start(sacc_out.ap()[:, :BT - 1], sacc_all[:, :BT - 1])
            nc.gpsimd.dma_start(sacc_out.ap()[:, 33:], sacc_all[:, 33:])
            i = BT - 1
            emit(i, (0, 1), i, 0)
            emit(i, (2, 3), 32, 0)
            emit(i, (4, 5), i, 1)
            emit(i, (6, 7), 32, 1)

        nc.sync.dma_start(sacc_out.ap()[:, BT - 1:33], sacc_all[:, BT - 1:33])

    nc.compile()
    return nc


def _get_nc():
    if "nc" not in _CACHE:
        _CACHE["nc"] = _build_nc()
    return _CACHE["nc"]


def _tile_slices(aT):
    """[D, N] (d-major) -> [NSL, P, 2, K2, 512] ko-parity-major slices.

    out[s, p, r, k2, b] = aT[(2*k2 + r)*128 + p, 512*s + b]
    """
    n = aT.shape[1]
    a = aT.reshape(K2, 2, P, n // 512, 512)        # [k2, r, p, s, b]
    return np.ascontiguousarray(a.transpose(3, 2, 1, 0, 4))


def _prep(inputs, corrected_targets, features):
    import concourse.mybir as mybir
    fp8 = mybir.dt.np(mybir.dt.float8e4)
    x = np.asarray(inputs, dtype=np.float32)
    f = np.asarray(features, dtype=np.float32)
    ct = np.asarray(corrected_targets).astype(np.int64)

    xn = x / np.linalg.norm(x, axis=1, keepdims=True)
    tdot = np.einsum('bd,bd->b', xn.astype(np.float64),
                     f[ct].astype(np.float64)) / TEMP

    xt = _tile_slices(np.ascontiguousarray((xn * SCALE).T)).astype(fp8)
    fT = np.ascontiguousarray((f * SCALE).T)                  # [D, NTOT]
    in_maps = []
    for c in range(NCORES):
        in_maps.append({
            "xt": xt,
            "ft": _tile_slices(fT[:, c * NS:(c + 1) * NS]).astype(fp8),
        })
    return in_maps, tdot


def _combine(results, tdot):
    S = np.zeros(B, dtype=np.float64)
    for c in range(NCORES):
        sacc = results[c]["sacc"].astype(np.float64)
        part = sacc[:, :BT].sum(axis=2)          # [P, BT]
        # partial-group accums (all jj=0 slots of their extra columns):
        # tiles 0-3: quarter j=1 in 34..37, half h1 in 38..41;
        # tiles 4-7: half h1 in 42..45; tiles 30/31 extras in 33/32 (both jj).
        part[:, 0:4] += sacc[:, 34:38, 0] + sacc[:, 38:42, 0]
        part[:, 4:8] += sacc[:, 42:46, 0]
        part[:, BT - 2] += sacc[:, 33, :].sum(axis=1)
        part[:, BT - 1] += sacc[:, 32, :].sum(axis=1)
        S += part.T.ravel()
    loss = np.mean(np.log(S) - tdot)
    return np.asarray(loss, dtype=np.float32)


def _run(inputs, targets, corrected_targets, features, trace=False, tmpdir=None):
    import time
    from concourse import bass_utils
    nc = _get_nc()
    in_maps, tdot = _prep(inputs, corrected_targets, features)
    last_exc = None
    for attempt in range(3):
        try:
            res = bass_utils.run_bass_kernel_spmd(
                nc, in_maps, core_ids=list(range(NCORES)), trace=trace,
                tmpdir=tmpdir)
            return _combine(res.results, tdot), res
        except Exception as e:  # transient device state (e.g. prior crash)
            last_exc = e
            time.sleep(2.0)
    raise last_exc


def kernel(inputs, targets, corrected_targets, features):
    out, _ = _run(inputs, targets, corrected_targets, features, trace=False)
    return out
